# revision 1
# baseline (speedup 1.0000x reference)
"""2D Haar DWT (DWT_2D) Trainium2 Bass kernel.

Input:  input [8, 64, 512, 512] f32 plus the four Haar DWT matrices.
Output: (LL, LH, HL, HH), each [8, 64, 256, 256] f32.

The Haar matrices have exactly two nonzeros (+-1/sqrt(2)) per row/col, so the
whole DWT is a 2x2 butterfly per input block:
    LL = 0.5*(a+b+c+d), LH = 0.5*(a-b+c-d),
    HL = 0.5*(a+b-c-d), HH = 0.5*(a-b-c+d)
with a=x[2i,2j], b=x[2i,2j+1], c=x[2i+1,2j], d=x[2i+1,2j+1]. The 0.5 scale is
folded into the host-side shard copy (exact in fp32), and the reference's
last-row/last-col zero quirks (Hh row 255, mh1 col 255) are applied on the
host after the gather — the device does pure adds/subs plus DMA.

Sharding: data-parallel over the batch dim, one batch element (64 slices of
[512,512]) per NeuronCore. Device kernel processes 2 slices per iteration:
one contiguous 2MB in-DMA, vertical butterfly on DVE, horizontal butterflies
split DVE/GpSimd into a single merged output tile, one 2MB out-DMA.
"""

import math
import os

import numpy as np

import concourse.bacc as bacc
import concourse.bass as bass
import concourse.mybir as mybir
from concourse.bass_utils import run_bass_kernel_spmd
from concourse.tile import TileContext

B, C, H, W = 8, 64, 512, 512
N_CORES = 8
SLICES_PER_CORE = (B * C) // N_CORES  # 64 [512,512] slices per core
PAIR = 2  # slices per device iteration
FP = mybir.dt.float32

_prog_cache = {}

# Set by test/profiling harnesses: when True, run_bass_kernel_spmd captures an
# NTFF profile and the BassKernelResults lands in LAST_RESULTS.
TRACE = False
LAST_RESULTS = None


def _build_program(n_slices: int) -> bass.Bass:
    # Bacc (not raw Bass): its compile() pass converts the Tile exit drain's
    # many sem waits into event semaphores; raw Bass fails walrus codegen
    # with "Too many sync wait commands".
    nc = bacc.Bacc(None, target_bir_lowering=False)
    x = nc.dram_tensor("x", [n_slices, H, W], FP, kind="ExternalInput")
    # All four subbands in one output tensor: [band, slice, 256, 256].
    out = nc.dram_tensor(
        "out", [4, n_slices, H // 2, W // 2], FP, kind="ExternalOutput"
    )

    n_iter = n_slices // PAIR
    # Input: 2 slices = 1024 rows; partition p holds rows 8p..8p+7 (4 row
    # pairs -> combined output rows 4p..4p+3). One contiguous 2MB DMA.
    x2 = x[:].rearrange("(i a) h w -> i (a h) w", a=PAIR)  # [i, 1024, 512]
    # Output: combined out row R = 4p+t; slice = R//256 (p>=64 -> 2nd slice).
    ov = out[:].rearrange(
        "b (i a) (pp t) w -> i (a pp) b t w", a=PAIR, t=4
    )  # [i, 128, 4, 4, 256]

    with TileContext(nc) as tc:
        with tc.tile_pool(name="pool", bufs=3) as pool:
            for i in range(n_iter):
                xt = pool.tile([128, 8, 512], FP, tag="xt", bufs=3)
                # In-DMAs on the Sync sequencer; out-DMAs on the (otherwise
                # idle) Scalar sequencer so out-DMA waits can't
                # head-of-line-block in-DMA issue.
                nc.sync.dma_start(
                    out=xt[:], in_=x2[i].rearrange("(p q) w -> p q w", p=128)
                )

                xe = xt[:, 0:8:2, :]  # even rows of the four pairs
                xo = xt[:, 1:8:2, :]  # odd rows
                st = pool.tile([128, 4, 512], FP, tag="st")  # vertical sum
                dt = pool.tile([128, 4, 512], FP, tag="dt")  # vertical diff
                nc.vector.tensor_add(out=st[:], in0=xe, in1=xo)
                nc.vector.tensor_sub(out=dt[:], in0=xe, in1=xo)

                s0 = st[:, :, 0:512:2]
                s1 = st[:, :, 1:512:2]
                d0 = dt[:, :, 0:512:2]
                d1 = dt[:, :, 1:512:2]

                # Output tiles split by producer: DVE writes LL/LH, GpSimd
                # writes HL/HH. Each ships independently. (Measured: DVE was
                # 92% busy vs Pool 44% when DVE also took half of HH.)
                oa = pool.tile([128, 2, 4, 256], FP, tag="oa")
                ob = pool.tile([128, 2, 4, 256], FP, tag="ob")
                nc.vector.tensor_add(out=oa[:, 0], in0=s0, in1=s1)  # LL
                nc.vector.tensor_sub(out=oa[:, 1], in0=s0, in1=s1)  # LH
                nc.gpsimd.tensor_add(out=ob[:, 0], in0=d0, in1=d1)  # HL
                nc.gpsimd.tensor_sub(out=ob[:, 1], in0=d0, in1=d1)  # HH

                nc.scalar.dma_start(out=ov[i][:, 0:2], in_=oa[:])
                nc.scalar.dma_start(out=ov[i][:, 2:4], in_=ob[:])
    nc.finalize()
    return nc


def _get_program(n_slices: int) -> bass.Bass:
    if n_slices not in _prog_cache:
        _prog_cache[n_slices] = _build_program(n_slices)
    return _prog_cache[n_slices]


def _expected_matrices():
    """Numpy port of reference.build_dwt_matrices for Haar, H=W=512."""
    sq = 1.0 / math.sqrt(2.0)
    ml0 = np.zeros((256, 512), np.float32)
    mh0 = np.zeros((256, 512), np.float32)
    for i in range(256):
        ml0[i, 2 * i : 2 * i + 2] = [sq, sq]
    for i in range(255):  # last row left zero (reference quirk)
        mh0[i, 2 * i : 2 * i + 2] = [sq, -sq]
    return ml0, ml0.T.copy(), mh0, mh0.T.copy()


def _numpy_fallback(x, ml0, ml1, mh0, mh1):
    out = []
    l = np.einsum("ih,bchw->bciw", ml0, x, optimize=True)
    hh_ = np.einsum("ih,bchw->bciw", mh0, x, optimize=True)
    for m in (l, hh_):
        for right in (ml1, mh1):
            out.append(np.einsum("bciw,wj->bcij", m, right, optimize=True))
    return tuple(np.ascontiguousarray(o.astype(np.float32)) for o in out)


def kernel(**inputs):
    x = np.asarray(inputs["input"], dtype=np.float32)
    assert x.shape == (B, C, H, W), x.shape

    ml0 = np.asarray(inputs["matrix_low_0"], dtype=np.float32)
    ml1 = np.asarray(inputs["matrix_low_1"], dtype=np.float32)
    mh0 = np.asarray(inputs["matrix_high_0"], dtype=np.float32)
    mh1 = np.asarray(inputs["matrix_high_1"], dtype=np.float32)
    el0, el1, eh0, eh1 = _expected_matrices()
    if not (
        np.array_equal(ml0, el0)
        and np.array_equal(ml1, el1)
        and np.array_equal(mh0, eh0)
        and np.array_equal(mh1, eh1)
    ):
        # Unexpected (non-Haar) matrices: stay correct via numpy.
        return _numpy_fallback(x, ml0, ml1, mh0, mh1)

    nc = _get_program(SLICES_PER_CORE)
    xs = x.reshape(B * C, H, W)
    # The 0.5 DWT scale rides on the per-core shard copy (exact in fp32).
    in_maps = [
        {"x": 0.5 * xs[i * SLICES_PER_CORE : (i + 1) * SLICES_PER_CORE]}
        for i in range(N_CORES)
    ]
    global LAST_RESULTS
    try:
        res = run_bass_kernel_spmd(
            nc, in_maps, core_ids=list(range(N_CORES)), trace=TRACE
        )
    except ModuleNotFoundError:
        # A stray BASS_TRACE=1 in the environment routes through the NTFF
        # hook import, which this image lacks — retry untraced.
        os.environ["BASS_NEVER_TRACE"] = "1"
        res = run_bass_kernel_spmd(
            nc, in_maps, core_ids=list(range(N_CORES)), trace=False
        )
    LAST_RESULTS = res
    full = np.concatenate(
        [res.results[i]["out"] for i in range(N_CORES)], axis=1
    ).reshape(4, B, C, H // 2, W // 2)
    ll, lh, hl, hh = full[0], full[1], full[2], full[3]
    # Reference quirks: Hh row 255 == 0 (HL/HH row 255), mh1 col 255 == 0
    # (LH/HH col 255).
    lh[..., :, 255] = 0.0
    hl[..., 255, :] = 0.0
    hh[..., 255, :] = 0.0
    hh[..., :, 255] = 0.0
    return (ll, lh, hl, hh)



# revision 2
# speedup vs baseline: 1.9326x; 1.9326x over previous
"""2D Haar DWT (DWT_2D) Trainium2 Bass kernel.

Input:  input [8, 64, 512, 512] f32 plus the four Haar DWT matrices.
Output: (LL, LH, HL, HH), each [8, 64, 256, 256] f32.

The Haar matrices have exactly two nonzeros (+-1/sqrt(2)) per row/col, so the
whole DWT is a 2x2 butterfly per input block:
    LL = 0.5*(a+b+c+d), LH = 0.5*(a-b+c-d),
    HL = 0.5*(a+b-c-d), HH = 0.5*(a-b-c+d)
with a=x[2i,2j], b=x[2i,2j+1], c=x[2i+1,2j], d=x[2i+1,2j+1].

The kernel is HBM-bandwidth bound, so all device I/O is fp16: the host folds
the 0.5 scale into the shard copy, de-interleaves even/odd columns (so the
horizontal butterfly reads step-1 APs and DVE stays in 2x perf mode -- fp32
or strided tensor_tensor would fall back to 1x), and casts to fp16; outputs
come back fp16 and are upcast on the host. fp16 round-trip error is ~1e-4
relative, far under the 2e-2 gate. The reference's last-row/last-col zero
quirks (Hh row 255, mh1 col 255) are applied on the host after the gather.

Sharding: data-parallel over the batch dim, one batch element (64 slices of
[512,512]) per NeuronCore. Device kernel processes 4 slices per iteration:
one contiguous 2MB in-DMA, vertical butterfly + 3 horizontal butterflies on
DVE, the last horizontal butterfly on GpSimd, two 1MB out-DMAs.
"""

import math
import os

import numpy as np

import concourse.bacc as bacc
import concourse.bass as bass
import concourse.mybir as mybir
from concourse.bass_utils import run_bass_kernel_spmd
from concourse.tile import TileContext

B, C, H, W = 8, 64, 512, 512
N_CORES = 8
SLICES_PER_CORE = (B * C) // N_CORES  # 64 [512,512] slices per core
PAIR = 4  # slices per device iteration
HP = mybir.dt.float16

_prog_cache = {}

# Set by test/profiling harnesses: when True, run_bass_kernel_spmd captures an
# NTFF profile and the BassKernelResults lands in LAST_RESULTS.
TRACE = False
LAST_RESULTS = None


def _build_program(n_slices: int) -> bass.Bass:
    # Bacc (not raw Bass): its compile() pass converts the Tile exit drain's
    # many sem waits into event semaphores; raw Bass fails walrus codegen
    # with "Too many sync wait commands".
    nc = bacc.Bacc(None, target_bir_lowering=False)
    # x rows are column-de-interleaved on the host: [even 256 | odd 256].
    x = nc.dram_tensor("x", [n_slices, H, W], HP, kind="ExternalInput")
    # All four subbands in one output tensor: [band, slice, 256, 256].
    out = nc.dram_tensor(
        "out", [4, n_slices, H // 2, W // 2], HP, kind="ExternalOutput"
    )

    n_iter = n_slices // PAIR
    # Input: 4 slices = 2048 rows; partition p holds rows 16p..16p+15 (8 row
    # pairs -> combined output rows 8p..8p+7). One contiguous 2MB DMA.
    x2 = x[:].rearrange("(i a) h w -> i (a h) w", a=PAIR)  # [i, 2048, 512]
    # Output: combined out row R = 8p+t; slice = R//256 (p = 32a+pp -> a'th
    # slice of the 4-block).
    ov = out[:].rearrange(
        "b (i a) (pp t) w -> i (a pp) b t w", a=PAIR, t=8
    )  # [i, 128, 4, 8, 256]

    with TileContext(nc) as tc:
        with tc.tile_pool(name="pool", bufs=3) as pool:
            for i in range(n_iter):
                xt = pool.tile([128, 16, 512], HP, tag="xt", bufs=3)
                # In-DMAs on the Sync sequencer; out-DMAs on the (otherwise
                # idle) Scalar sequencer so out-DMA waits can't
                # head-of-line-block in-DMA issue.
                nc.sync.dma_start(
                    out=xt[:], in_=x2[i].rearrange("(p q) w -> p q w", p=128)
                )

                xe = xt[:, 0:16:2, :]  # even rows of the eight pairs
                xo = xt[:, 1:16:2, :]  # odd rows
                st = pool.tile([128, 8, 512], HP, tag="st")  # vertical sum
                dt = pool.tile([128, 8, 512], HP, tag="dt")  # vertical diff
                nc.vector.tensor_add(out=st[:], in0=xe, in1=xo)
                nc.vector.tensor_sub(out=dt[:], in0=xe, in1=xo)

                # Host de-interleave put even columns in [0:256), odd in
                # [256:512) -- all horizontal butterflies are step-1.
                s0 = st[:, :, 0:256]
                s1 = st[:, :, 256:512]
                d0 = dt[:, :, 0:256]
                d1 = dt[:, :, 256:512]

                oa = pool.tile([128, 2, 8, 256], HP, tag="oa")
                ob = pool.tile([128, 2, 8, 256], HP, tag="ob")
                nc.vector.tensor_add(out=oa[:, 0], in0=s0, in1=s1)  # LL
                nc.vector.tensor_sub(out=oa[:, 1], in0=s0, in1=s1)  # LH
                nc.vector.tensor_add(out=ob[:, 0], in0=d0, in1=d1)  # HL
                nc.gpsimd.tensor_sub(out=ob[:, 1], in0=d0, in1=d1)  # HH

                nc.scalar.dma_start(out=ov[i][:, 0:2], in_=oa[:])
                nc.scalar.dma_start(out=ov[i][:, 2:4], in_=ob[:])
    nc.finalize()
    return nc


def _get_program(n_slices: int) -> bass.Bass:
    if n_slices not in _prog_cache:
        _prog_cache[n_slices] = _build_program(n_slices)
    return _prog_cache[n_slices]


def _expected_matrices():
    """Numpy port of reference.build_dwt_matrices for Haar, H=W=512."""
    sq = 1.0 / math.sqrt(2.0)
    ml0 = np.zeros((256, 512), np.float32)
    mh0 = np.zeros((256, 512), np.float32)
    for i in range(256):
        ml0[i, 2 * i : 2 * i + 2] = [sq, sq]
    for i in range(255):  # last row left zero (reference quirk)
        mh0[i, 2 * i : 2 * i + 2] = [sq, -sq]
    return ml0, ml0.T.copy(), mh0, mh0.T.copy()


def _numpy_fallback(x, ml0, ml1, mh0, mh1):
    out = []
    l = np.einsum("ih,bchw->bciw", ml0, x, optimize=True)
    hh_ = np.einsum("ih,bchw->bciw", mh0, x, optimize=True)
    for m in (l, hh_):
        for right in (ml1, mh1):
            out.append(np.einsum("bciw,wj->bcij", m, right, optimize=True))
    return tuple(np.ascontiguousarray(o.astype(np.float32)) for o in out)


def kernel(**inputs):
    x = np.asarray(inputs["input"], dtype=np.float32)
    assert x.shape == (B, C, H, W), x.shape

    ml0 = np.asarray(inputs["matrix_low_0"], dtype=np.float32)
    ml1 = np.asarray(inputs["matrix_low_1"], dtype=np.float32)
    mh0 = np.asarray(inputs["matrix_high_0"], dtype=np.float32)
    mh1 = np.asarray(inputs["matrix_high_1"], dtype=np.float32)
    el0, el1, eh0, eh1 = _expected_matrices()
    if not (
        np.array_equal(ml0, el0)
        and np.array_equal(ml1, el1)
        and np.array_equal(mh0, eh0)
        and np.array_equal(mh1, eh1)
    ):
        # Unexpected (non-Haar) matrices: stay correct via numpy.
        return _numpy_fallback(x, ml0, ml1, mh0, mh1)

    nc = _get_program(SLICES_PER_CORE)
    # Per core shard prep: 0.5 DWT scale (exact), even/odd column
    # de-interleave, fp16 cast. [64, 512, 256, 2] -> [64, 512, 2, 256].
    xs = x.reshape(B * C, H, W // 2, 2)
    in_maps = []
    for i in range(N_CORES):
        shard = xs[i * SLICES_PER_CORE : (i + 1) * SLICES_PER_CORE]
        buf = (shard.transpose(0, 1, 3, 2) * np.float32(0.5)).astype(np.float16)
        in_maps.append({"x": np.ascontiguousarray(buf).reshape(SLICES_PER_CORE, H, W)})
    global LAST_RESULTS
    try:
        res = run_bass_kernel_spmd(
            nc, in_maps, core_ids=list(range(N_CORES)), trace=TRACE
        )
    except ModuleNotFoundError:
        # A stray BASS_TRACE=1 in the environment routes through the NTFF
        # hook import, which this image lacks — retry untraced.
        os.environ["BASS_NEVER_TRACE"] = "1"
        res = run_bass_kernel_spmd(
            nc, in_maps, core_ids=list(range(N_CORES)), trace=False
        )
    LAST_RESULTS = res
    full = np.concatenate(
        [res.results[i]["out"] for i in range(N_CORES)], axis=1
    ).reshape(4, B, C, H // 2, W // 2)
    full = full.astype(np.float32)
    ll, lh, hl, hh = full[0], full[1], full[2], full[3]
    # Reference quirks: Hh row 255 == 0 (HL/HH row 255), mh1 col 255 == 0
    # (LH/HH col 255).
    lh[..., :, 255] = 0.0
    hl[..., 255, :] = 0.0
    hh[..., 255, :] = 0.0
    hh[..., :, 255] = 0.0
    return (ll, lh, hl, hh)


# revision 3
# speedup vs baseline: 2.0423x; 1.0568x over previous
"""2D Haar DWT (DWT_2D) Trainium2 Bass kernel.

Input:  input [8, 64, 512, 512] f32 plus the four Haar DWT matrices.
Output: (LL, LH, HL, HH), each [8, 64, 256, 256] f32.

The Haar matrices have exactly two nonzeros (+-1/sqrt(2)) per row/col, so the
whole DWT is a 2x2 butterfly per input block:
    LL = 0.5*(a+b+c+d), LH = 0.5*(a-b+c-d),
    HL = 0.5*(a+b-c-d), HH = 0.5*(a-b-c+d)
with a=x[2i,2j], b=x[2i,2j+1], c=x[2i+1,2j], d=x[2i+1,2j+1].

The kernel is HBM-bandwidth bound, so all device I/O is fp16: the host folds
the 0.5 scale into the shard copy, de-interleaves even/odd columns (so the
horizontal butterfly reads step-1 APs and DVE stays in 2x perf mode -- fp32
or strided tensor_tensor would fall back to 1x), and casts to fp16; outputs
come back fp16 and are upcast on the host. fp16 round-trip error is ~1e-4
relative, far under the 2e-2 gate. The reference's last-row/last-col zero
quirks (Hh row 255, mh1 col 255) are applied on the host after the gather.

Sharding: data-parallel over the batch dim, one batch element (64 slices of
[512,512]) per NeuronCore. Device kernel processes 4 slices per iteration:
one contiguous 2MB in-DMA, vertical butterfly + 3 horizontal butterflies on
DVE, the last horizontal butterfly on GpSimd, two 1MB out-DMAs.
"""

import math
import os

import numpy as np

import concourse.bacc as bacc
import concourse.bass as bass
import concourse.mybir as mybir
from concourse.bass_utils import run_bass_kernel_spmd
from concourse.tile import TileContext

B, C, H, W = 8, 64, 512, 512
N_CORES = 8
SLICES_PER_CORE = (B * C) // N_CORES  # 64 [512,512] slices per core
PAIR = 4  # slices per device iteration
HP = mybir.dt.float16

_prog_cache = {}

# Set by test/profiling harnesses: when True, run_bass_kernel_spmd captures an
# NTFF profile and the BassKernelResults lands in LAST_RESULTS.
TRACE = False
LAST_RESULTS = None


def _build_program(n_slices: int) -> bass.Bass:
    # Bacc (not raw Bass): its compile() pass converts the Tile exit drain's
    # many sem waits into event semaphores; raw Bass fails walrus codegen
    # with "Too many sync wait commands".
    nc = bacc.Bacc(None, target_bir_lowering=False)
    # x rows are column-de-interleaved on the host: [even 256 | odd 256].
    x = nc.dram_tensor("x", [n_slices, H, W], HP, kind="ExternalInput")
    # All four subbands in one output tensor: [band, slice, 256, 256].
    out = nc.dram_tensor(
        "out", [4, n_slices, H // 2, W // 2], HP, kind="ExternalOutput"
    )

    n_iter = n_slices // PAIR
    # Input: 4 slices = 2048 rows; partition p holds rows 16p..16p+15 (8 row
    # pairs -> combined output rows 8p..8p+7). One contiguous 2MB DMA.
    x2 = x[:].rearrange("(i a) h w -> i (a h) w", a=PAIR)  # [i, 2048, 512]
    # Output: combined out row R = 8p+t; slice = R//256 (p = 32a+pp -> a'th
    # slice of the 4-block).
    ov = out[:].rearrange(
        "b (i a) (pp t) w -> i (a pp) b t w", a=PAIR, t=8
    )  # [i, 128, 4, 8, 256]

    with TileContext(nc) as tc:
        with tc.tile_pool(name="pool", bufs=3) as pool:
            for i in range(n_iter):
                xt = pool.tile([128, 16, 512], HP, tag="xt", bufs=3)
                # In-DMAs on the Sync sequencer; out-DMAs on the (otherwise
                # idle) Scalar sequencer so out-DMA waits can't
                # head-of-line-block in-DMA issue.
                nc.sync.dma_start(
                    out=xt[:], in_=x2[i].rearrange("(p q) w -> p q w", p=128)
                )

                xe = xt[:, 0:16:2, :]  # even rows of the eight pairs
                xo = xt[:, 1:16:2, :]  # odd rows
                st = pool.tile([128, 8, 512], HP, tag="st")  # vertical sum
                dt = pool.tile([128, 8, 512], HP, tag="dt")  # vertical diff
                nc.vector.tensor_add(out=st[:], in0=xe, in1=xo)
                nc.vector.tensor_sub(out=dt[:], in0=xe, in1=xo)

                # Host de-interleave put even columns in [0:256), odd in
                # [256:512) -- all horizontal butterflies are step-1.
                s0 = st[:, :, 0:256]
                s1 = st[:, :, 256:512]
                d0 = dt[:, :, 0:256]
                d1 = dt[:, :, 256:512]

                oa = pool.tile([128, 2, 8, 256], HP, tag="oa")
                ob = pool.tile([128, 2, 8, 256], HP, tag="ob")
                # All butterflies on DVE: every DVE tensor_tensor needs the
                # shared SBUF port pair for src1, and GpSimd ops grab that
                # same pair under an exclusive whole-instruction lock -- a
                # "parallel" GpSimd op just serializes with (and blocks) DVE.
                # DVE alone fits under the per-iteration DMA budget.
                nc.vector.tensor_add(out=oa[:, 0], in0=s0, in1=s1)  # LL
                nc.vector.tensor_sub(out=oa[:, 1], in0=s0, in1=s1)  # LH
                nc.vector.tensor_add(out=ob[:, 0], in0=d0, in1=d1)  # HL
                nc.vector.tensor_sub(out=ob[:, 1], in0=d0, in1=d1)  # HH

                nc.scalar.dma_start(out=ov[i][:, 0:2], in_=oa[:])
                nc.scalar.dma_start(out=ov[i][:, 2:4], in_=ob[:])
    nc.finalize()
    return nc


def _get_program(n_slices: int) -> bass.Bass:
    if n_slices not in _prog_cache:
        _prog_cache[n_slices] = _build_program(n_slices)
    return _prog_cache[n_slices]


def _expected_matrices():
    """Numpy port of reference.build_dwt_matrices for Haar, H=W=512."""
    sq = 1.0 / math.sqrt(2.0)
    ml0 = np.zeros((256, 512), np.float32)
    mh0 = np.zeros((256, 512), np.float32)
    for i in range(256):
        ml0[i, 2 * i : 2 * i + 2] = [sq, sq]
    for i in range(255):  # last row left zero (reference quirk)
        mh0[i, 2 * i : 2 * i + 2] = [sq, -sq]
    return ml0, ml0.T.copy(), mh0, mh0.T.copy()


def _numpy_fallback(x, ml0, ml1, mh0, mh1):
    out = []
    l = np.einsum("ih,bchw->bciw", ml0, x, optimize=True)
    hh_ = np.einsum("ih,bchw->bciw", mh0, x, optimize=True)
    for m in (l, hh_):
        for right in (ml1, mh1):
            out.append(np.einsum("bciw,wj->bcij", m, right, optimize=True))
    return tuple(np.ascontiguousarray(o.astype(np.float32)) for o in out)


def kernel(**inputs):
    x = np.asarray(inputs["input"], dtype=np.float32)
    assert x.shape == (B, C, H, W), x.shape

    ml0 = np.asarray(inputs["matrix_low_0"], dtype=np.float32)
    ml1 = np.asarray(inputs["matrix_low_1"], dtype=np.float32)
    mh0 = np.asarray(inputs["matrix_high_0"], dtype=np.float32)
    mh1 = np.asarray(inputs["matrix_high_1"], dtype=np.float32)
    el0, el1, eh0, eh1 = _expected_matrices()
    if not (
        np.array_equal(ml0, el0)
        and np.array_equal(ml1, el1)
        and np.array_equal(mh0, eh0)
        and np.array_equal(mh1, eh1)
    ):
        # Unexpected (non-Haar) matrices: stay correct via numpy.
        return _numpy_fallback(x, ml0, ml1, mh0, mh1)

    nc = _get_program(SLICES_PER_CORE)
    # Per core shard prep: 0.5 DWT scale (exact), even/odd column
    # de-interleave, fp16 cast. [64, 512, 256, 2] -> [64, 512, 2, 256].
    xs = x.reshape(B * C, H, W // 2, 2)
    in_maps = []
    for i in range(N_CORES):
        shard = xs[i * SLICES_PER_CORE : (i + 1) * SLICES_PER_CORE]
        buf = (shard.transpose(0, 1, 3, 2) * np.float32(0.5)).astype(np.float16)
        in_maps.append({"x": np.ascontiguousarray(buf).reshape(SLICES_PER_CORE, H, W)})
    global LAST_RESULTS
    try:
        res = run_bass_kernel_spmd(
            nc, in_maps, core_ids=list(range(N_CORES)), trace=TRACE
        )
    except ModuleNotFoundError:
        # A stray BASS_TRACE=1 in the environment routes through the NTFF
        # hook import, which this image lacks — retry untraced.
        os.environ["BASS_NEVER_TRACE"] = "1"
        res = run_bass_kernel_spmd(
            nc, in_maps, core_ids=list(range(N_CORES)), trace=False
        )
    LAST_RESULTS = res
    full = np.concatenate(
        [res.results[i]["out"] for i in range(N_CORES)], axis=1
    ).reshape(4, B, C, H // 2, W // 2)
    full = full.astype(np.float32)
    ll, lh, hl, hh = full[0], full[1], full[2], full[3]
    # Reference quirks: Hh row 255 == 0 (HL/HH row 255), mh1 col 255 == 0
    # (LH/HH col 255).
    lh[..., :, 255] = 0.0
    hl[..., 255, :] = 0.0
    hh[..., 255, :] = 0.0
    hh[..., :, 255] = 0.0
    return (ll, lh, hl, hh)
